# revision 1
# baseline (speedup 1.0000x reference)
"""Block-diagonal linear for TRN2, 8 NeuronCores.

y = concat_h(x_h @ w_h + b_h) with x:[4,4096,4096] split into 16 blocks of
256 features; w:[16,256,256]; b:[16,256].

Sharding: data-parallel over rows. x is reshaped to [16384, 4096] and each
core takes 2048 contiguous rows; w and b are replicated. Zero communication.

Per-core kernel (Tile framework):
  - w is staged in SBUF as [128, 16, 2, 256] (contraction dim on partitions).
  - b is broadcast across partitions once via gpsimd partition_broadcast.
  - For each 128-row tile of x: DMA in naturally (4 chunks), then per group of
    4 feature-chunks: PE-transpose them (features onto partitions, bit-exact
    fp32 transpose via identity matmul), copy PSUM->SBUF on ACT, and run the
    2-step accumulated fp32 matmuls for the 2 feature blocks they feed; DVE
    adds the bias while evicting PSUM->SBUF; y DMAs out in 4 chunks.
  - Everything fp32: exact same arithmetic as the reference (PE fp32 matmul
    is bit-accurate per-product with fp32 PSUM accumulation).
"""

import numpy as np

import concourse.bacc as bacc
import concourse.mybir as mybir
from concourse import bass2jax, tile
from concourse.masks import make_identity

N_CORES = 8
ROWS_TOTAL = 4 * 4096
ROWS = ROWS_TOTAL // N_CORES  # 2048 rows per core
WIDTH = 4096
NB = 16  # feature blocks
BW = 256  # block width
P = 128
M_TILES = ROWS // P  # 16

FP32 = mybir.dt.float32


def _build(repeat=1, xb=2, xtb=2, yb=2, ptb=3, pyb=5):
    nc = bacc.Bacc(None, target_bir_lowering=False, debug=False)
    x = nc.dram_tensor("x", [ROWS, WIDTH], FP32, kind="ExternalInput")
    w = nc.dram_tensor("w", [NB, BW, BW], FP32, kind="ExternalInput")
    b = nc.dram_tensor("b", [NB, BW], FP32, kind="ExternalInput")
    y = nc.dram_tensor("y", [ROWS, WIDTH], FP32, kind="ExternalOutput")

    with tile.TileContext(nc) as tc:
        with (
            tc.tile_pool(name="const", bufs=1) as const_pool,
            tc.tile_pool(name="xpool", bufs=xb) as x_pool,
            tc.tile_pool(name="xtpool", bufs=xtb) as xt_pool,
            tc.tile_pool(name="ypool", bufs=yb) as y_pool,
            tc.tile_pool(name="pt", bufs=ptb, space="PSUM") as psum_t,
            tc.tile_pool(name="py", bufs=pyb, space="PSUM") as psum_y,
        ):
            ident = const_pool.tile([P, P], FP32)
            make_identity(nc, ident[:])

            # Weights + bias on the ACT HWDGE ring so they don't head-block
            # the x-tile loads issued on the SP ring.
            # w_sb[p, h, ic, j] = w[h, ic*128+p, j]; staged in 4 chunks so the
            # first matmuls only wait for blocks 0-3.
            w_sb = const_pool.tile([P, NB, 2, BW], FP32)
            w_re = w.rearrange("h (ic p) j -> p h ic j", p=P)
            for q in range(4):
                nc.scalar.dma_start(
                    w_sb[:, 4 * q:4 * q + 4], w_re[:, 4 * q:4 * q + 4]
                )

            # Bias broadcast across partitions on GpSimd (Pool engine):
            # b_rep[p, h, j] = b[h, j].
            b_lin = const_pool.tile([1, NB, BW], FP32)
            nc.scalar.dma_start(
                b_lin[:], b.rearrange("(one h) j -> one h j", one=1)
            )
            b_rep = const_pool.tile([P, NB, BW], FP32)
            nc.gpsimd.partition_broadcast(
                b_rep[:].rearrange("p h j -> p (h j)"),
                b_lin[:].rearrange("o h j -> o (h j)"),
            )

            x_rows = x.rearrange("(t p) i -> t p i", p=P)
            y_rows = y.rearrange("(t p) i -> t p i", p=P)

            import contextlib

            rep_ctx = (
                tc.For_i(0, repeat, 1) if repeat > 1 else contextlib.nullcontext()
            )
            with rep_ctx:
                _main_loop(nc, tc, locals())

    nc.compile()
    return nc


def _main_loop(nc, tc, env):
    x_pool = env["x_pool"]
    xt_pool = env["xt_pool"]
    y_pool = env["y_pool"]
    psum_t = env["psum_t"]
    psum_y = env["psum_y"]
    ident = env["ident"]
    w_sb = env["w_sb"]
    b_rep = env["b_rep"]
    x_rows = env["x_rows"]
    y_rows = env["y_rows"]
    if True:
            for mi in range(M_TILES):
                x_t = x_pool.tile([P, WIDTH], FP32)
                qw = WIDTH // 4
                for q in range(4):
                    nc.sync.dma_start(
                        x_t[:, q * qw:(q + 1) * qw],
                        x_rows[mi][:, q * qw:(q + 1) * qw],
                    )

                # Per group g: transpose chunks 4g..4g+3 (features onto
                # partitions), then immediately the matmuls for blocks
                # 2g, 2g+1 which consume exactly those chunks. Interleaving
                # keeps real matmuls inside every HAM activity window.
                xT = xt_pool.tile([P, WIDTH // P, P], FP32)
                y_t = y_pool.tile([P, NB, BW], FP32)
                for g in range(8):
                    pt = psum_t.tile([P, 4, P], FP32, tag="pt")
                    for k in range(4):
                        c = 4 * g + k
                        nc.tensor.transpose(
                            pt[:, k, :], x_t[:, c * P:(c + 1) * P], ident[:]
                        )
                    nc.scalar.copy(xT[:, 4 * g:4 * g + 4, :], pt[:])

                    py = psum_y.tile([P, 2, BW], FP32)
                    for u in range(2):
                        h = 2 * g + u
                        nc.tensor.matmul(
                            py[:, u, :], xT[:, 2 * h, :], w_sb[:, h, 0, :],
                            start=True, stop=False,
                        )
                        nc.tensor.matmul(
                            py[:, u, :], xT[:, 2 * h + 1, :], w_sb[:, h, 1, :],
                            start=False, stop=True,
                        )
                    nc.vector.tensor_add(
                        y_t[:, 2 * g:2 * g + 2, :],
                        py[:],
                        b_rep[:, 2 * g:2 * g + 2, :],
                    )
                    if mi == M_TILES - 1:
                        # Last tile: stream each group's slice out right after
                        # its bias-add so the kernel tail is just one small
                        # DMA instead of a quarter-row.
                        nc.sync.dma_start(
                            y_rows[mi][:, g * 512:(g + 1) * 512],
                            y_t[:].rearrange("p h j -> p (h j)")[
                                :, g * 512:(g + 1) * 512
                            ],
                        )
                if mi != M_TILES - 1:
                    y_flat = y_t[:].rearrange("p h j -> p (h j)")
                    qw = WIDTH // 4
                    for q in range(4):
                        nc.sync.dma_start(
                            y_rows[mi][:, q * qw:(q + 1) * qw],
                            y_flat[:, q * qw:(q + 1) * qw],
                        )


class _Runner:
    """Compile once, keep the jitted SPMD executable for reuse."""

    def __init__(self, repeat=1):
        import jax
        from jax.experimental.shard_map import shard_map
        from jax.sharding import Mesh, PartitionSpec

        self.jax = jax
        nc = _build(repeat=repeat)
        bass2jax.install_neuronx_cc_hook()

        assert nc.dbg_addr is None
        part_name = (
            nc.partition_id_tensor.name if nc.partition_id_tensor else None
        )
        in_names, out_names, out_avals = [], [], []
        for alloc in nc.m.functions[0].allocations:
            if not isinstance(alloc, mybir.MemoryLocationSet):
                continue
            name = alloc.memorylocations[0].name
            if alloc.kind == "ExternalInput":
                if name != part_name:
                    in_names.append(name)
            elif alloc.kind == "ExternalOutput":
                out_names.append(name)
                out_avals.append(
                    jax.core.ShapedArray(
                        tuple(alloc.tensor_shape), mybir.dt.np(alloc.dtype)
                    )
                )
        self.in_names = list(in_names)
        self.out_names = out_names
        self.out_avals = out_avals
        n_params = len(in_names)
        n_outs = len(out_names)
        all_names = in_names + out_names
        if part_name is not None:
            all_names = all_names + [part_name]

        def _body(*args):
            operands = list(args)
            if part_name is not None:
                operands.append(bass2jax.partition_id_tensor())
            outs = bass2jax._bass_exec_p.bind(
                *operands,
                out_avals=tuple(out_avals),
                in_names=tuple(all_names),
                out_names=tuple(out_names),
                lowering_input_output_aliases=(),
                sim_require_finite=True,
                sim_require_nnan=True,
                nc=nc,
            )
            return tuple(outs)

        devices = jax.devices()[:N_CORES]
        assert len(devices) == N_CORES
        self.mesh = Mesh(np.asarray(devices), ("core",))
        in_specs = (PartitionSpec("core"),) * (n_params + n_outs)
        out_specs = (PartitionSpec("core"),) * n_outs
        self.donate = tuple(range(n_params, n_params + n_outs))
        self.fn = jax.jit(
            shard_map(
                _body,
                mesh=self.mesh,
                in_specs=in_specs,
                out_specs=out_specs,
                check_rep=False,
            ),
            donate_argnums=self.donate,
            keep_unused=True,
        )

    def zeros(self):
        return [
            np.zeros((N_CORES * a.shape[0], *a.shape[1:]), a.dtype)
            for a in self.out_avals
        ]

    def prep(self, x, w, b):
        """Global (concatenated-over-cores) input arrays, in in_names order."""
        x2 = np.ascontiguousarray(
            np.asarray(x, dtype=np.float32).reshape(ROWS_TOTAL, WIDTH)
        )
        w = np.ascontiguousarray(np.asarray(w, dtype=np.float32))
        b = np.ascontiguousarray(np.asarray(b, dtype=np.float32))
        per = {
            "x": x2,
            "w": np.concatenate([w] * N_CORES, axis=0),
            "b": np.concatenate([b] * N_CORES, axis=0),
        }
        return [per[n] for n in self.in_names]

    def __call__(self, ins, zeros):
        outs = self.fn(*ins, *zeros)
        return dict(zip(self.out_names, outs))


_RUNNER = None


def _get_runner():
    global _RUNNER
    if _RUNNER is None:
        _RUNNER = _Runner()
    return _RUNNER


def kernel(x, w, b):
    r = _get_runner()
    outs = r(r.prep(x, w, b), r.zeros())
    y = np.asarray(outs["y"])
    return y.reshape(4, 4096, WIDTH)



# revision 12
# speedup vs baseline: 418.0969x; 418.0969x over previous
"""Block-diagonal linear for TRN2, 8 NeuronCores.

y = concat_h(x_h @ w_h + b_h) with x:[4,4096,4096] split into 16 blocks of
256 features; w:[16,256,256]; b:[16,256].

Sharding: data-parallel over rows. x is reshaped to [16384, 4096] and each
core takes 2048 contiguous rows; w and b are replicated. Zero communication.
(A hybrid rows-x-feature-blocks split was measured ~25us slower despite
halving replicated-w traffic: the narrower per-core tiles shrink DMA runs
and make the y drain lag the tile stream.)

Per-core kernel (Tile framework), v3:
  - Matmuls run as float32r (single-pass PE streaming, 1 cyc/row at N=256 —
    4x the fp32 LOW_HIGH dual-pass rate; operands rounded to fp32r costs
    ~1.5e-4 relative error, well inside the 2e-2 gate).
  - w is staged in SBUF as [128, 8, 2, 256] fp32r (contraction on partitions)
    via the gpsimd SWDGE queue: its gather pattern needs ~2048 descriptors,
    which would head-of-line block an HWDGE sequencer (ACT blocked ~25us
    measured); Q7 generates them off the critical path.
  - b is broadcast across partitions once via gpsimd partition_broadcast.
  - Per 128-row tile: x DMAs in as 2 half-tiles (8KB/partition descriptors)
    on the SP HWDGE ring. Per group g of 4 feature-chunks: PE-transposes
    (fp32, bit-exact) land in PSUM, ACT evicts+rounds them to an fp32r xT
    buffer, and the 4 fp32r matmuls for blocks 2g,2g+1 consume them. The
    emit order is software-pipelined (transposes of group g+1 issue before
    the matmuls of group g) so the PE covers the ACT eviction latency with
    transpose work instead of stalling.
  - DVE adds bias while evicting PSUM->SBUF; y streams out per half-tile on
    the gpsimd (SWDGE) queue so output drains never head-of-line block the
    SP x-loads or the ACT evictions.
"""

import numpy as np

import concourse.bacc as bacc
import concourse.mybir as mybir
from concourse import bass2jax, tile
from concourse.masks import make_identity

N_CORES = 8
ROW_SHARDS = 8
FEAT_SHARDS = 1
ROWS_TOTAL = 4 * 4096
WIDTH_TOTAL = 4096
NB_TOTAL = 16
ROWS = ROWS_TOTAL // ROW_SHARDS  # 2048 rows per core
WIDTH = WIDTH_TOTAL // FEAT_SHARDS  # 4096 features per core
NB = NB_TOTAL // FEAT_SHARDS  # 16 blocks per core
BW = 256  # block width
P = 128
M_TILES = ROWS // P  # 16
NCH = WIDTH // P  # 32 feature chunks of 128
GROUPS = NCH // 4  # 8 groups of 4 chunks per tile

FP32 = mybir.dt.float32
FP32R = mybir.dt.float32r


def _build():
    nc = bacc.Bacc(None, target_bir_lowering=False, debug=False)
    x = nc.dram_tensor("x", [ROWS, WIDTH], FP32, kind="ExternalInput")
    w = nc.dram_tensor("w", [NB, BW, BW], FP32, kind="ExternalInput")
    b = nc.dram_tensor("b", [NB, BW], FP32, kind="ExternalInput")
    y = nc.dram_tensor("y", [ROWS, WIDTH], FP32, kind="ExternalOutput")

    with tile.TileContext(nc) as tc:
        with (
            tc.tile_pool(name="const", bufs=1) as const_pool,
            tc.tile_pool(name="xpool", bufs=3) as x_pool,
            tc.tile_pool(name="xtpool", bufs=2) as xt_pool,
            tc.tile_pool(name="ypool", bufs=2) as y_pool,
            tc.tile_pool(name="pt", bufs=3, space="PSUM") as psum_t,
            tc.tile_pool(name="py", bufs=5, space="PSUM") as psum_y,
        ):
            ident = const_pool.tile([P, P], FP32)
            make_identity(nc, ident[:])

            # Bias broadcast across partitions: b_rep[p, h, j] = b[h, j].
            b_lin = const_pool.tile([1, NB, BW], FP32)
            nc.gpsimd.dma_start(
                b_lin[:], b.rearrange("(one h) j -> one h j", one=1)
            )

            # w_sb[p, h, ic, j] = w[h, ic*128+p, j] (fp32r view: PE rounds
            # internally, bits are plain fp32). SWDGE queue: the gather
            # pattern's ~2048 descriptors would head-block an HWDGE ring.
            w_sb = const_pool.tile([P, NB, 2, BW], FP32R)
            w_re = w.rearrange("h (ic p) j -> p h ic j", p=P)
            for q in range(4):
                nc.gpsimd.dma_start(
                    w_sb[:, 4 * q:4 * q + 4],
                    w_re[:, 4 * q:4 * q + 4].bitcast(FP32R),
                )
            b_rep = const_pool.tile([P, NB, BW], FP32)
            nc.gpsimd.partition_broadcast(
                b_rep[:].rearrange("p h j -> p (h j)"),
                b_lin[:].rearrange("o h j -> o (h j)"),
            )

            x_rows = x.rearrange("(t p) i -> t p i", p=P)
            y_rows = y.rearrange("(t p) i -> t p i", p=P)
            HW = WIDTH // 2  # half-tile width

            for mi in range(M_TILES):
                x_t = x_pool.tile([P, WIDTH], FP32)
                for hf in range(2):
                    nc.sync.dma_start(
                        x_t[:, hf * HW:(hf + 1) * HW],
                        x_rows[mi][:, hf * HW:(hf + 1) * HW],
                    )

                xT = xt_pool.tile([P, NCH, P], FP32R)
                y_t = y_pool.tile([P, NB, BW], FP32)
                y_flat = y_t[:].rearrange("p h j -> p (h j)")

                # Software pipeline over the groups: transposes of group g+1
                # are emitted before the matmuls of group g so the PE has
                # transpose work while ACT evicts group g's PSUM.
                pts = {}

                def do_transposes(g):
                    pt = psum_t.tile([P, 4, P], FP32, tag="pt")
                    for k in range(4):
                        c = 4 * g + k
                        nc.tensor.transpose(
                            pt[:, k, :], x_t[:, c * P:(c + 1) * P], ident[:]
                        )
                    pts[g] = pt

                do_transposes(0)
                for g in range(GROUPS):
                    # ACT: evict + round group g's transposed chunks.
                    nc.scalar.copy(xT[:, 4 * g:4 * g + 4, :], pts.pop(g)[:])
                    if g < GROUPS - 1:
                        do_transposes(g + 1)

                    py = psum_y.tile([P, 2, BW], FP32)
                    for u in range(2):
                        h = 2 * g + u
                        nc.tensor.matmul(
                            py[:, u, :], xT[:, 2 * h, :], w_sb[:, h, 0, :],
                            start=True, stop=False,
                        )
                        nc.tensor.matmul(
                            py[:, u, :], xT[:, 2 * h + 1, :], w_sb[:, h, 1, :],
                            start=False, stop=True,
                        )
                    nc.vector.tensor_add(
                        y_t[:, 2 * g:2 * g + 2, :],
                        py[:],
                        b_rep[:, 2 * g:2 * g + 2, :],
                    )
                    if g == GROUPS // 2 - 1 or g == GROUPS - 1:
                        hf = 0 if g == GROUPS // 2 - 1 else 1
                        nc.gpsimd.dma_start(
                            y_rows[mi][:, hf * HW:(hf + 1) * HW],
                            y_flat[:, hf * HW:(hf + 1) * HW],
                        )

    nc.compile()
    return nc


class _Runner:
    """Compile once, keep the jitted SPMD executable for reuse."""

    def __init__(self):
        import jax
        from jax.experimental.shard_map import shard_map
        from jax.sharding import Mesh, PartitionSpec

        self.jax = jax
        nc = _build()
        self.nc = nc
        bass2jax.install_neuronx_cc_hook()

        assert nc.dbg_addr is None
        part_name = (
            nc.partition_id_tensor.name if nc.partition_id_tensor else None
        )
        in_names, out_names, out_avals = [], [], []
        for alloc in nc.m.functions[0].allocations:
            if not isinstance(alloc, mybir.MemoryLocationSet):
                continue
            name = alloc.memorylocations[0].name
            if alloc.kind == "ExternalInput":
                if name != part_name:
                    in_names.append(name)
            elif alloc.kind == "ExternalOutput":
                out_names.append(name)
                out_avals.append(
                    jax.core.ShapedArray(
                        tuple(alloc.tensor_shape), mybir.dt.np(alloc.dtype)
                    )
                )
        self.in_names = list(in_names)
        self.out_names = out_names
        self.out_avals = out_avals
        n_params = len(in_names)
        n_outs = len(out_names)
        all_names = in_names + out_names
        if part_name is not None:
            all_names = all_names + [part_name]

        def _body(*args):
            operands = list(args)
            if part_name is not None:
                operands.append(bass2jax.partition_id_tensor())
            outs = bass2jax._bass_exec_p.bind(
                *operands,
                out_avals=tuple(out_avals),
                in_names=tuple(all_names),
                out_names=tuple(out_names),
                lowering_input_output_aliases=(),
                sim_require_finite=True,
                sim_require_nnan=True,
                nc=nc,
            )
            return tuple(outs)

        devices = jax.devices()[:N_CORES]
        assert len(devices) == N_CORES
        self.mesh = Mesh(np.asarray(devices), ("core",))
        in_specs = (PartitionSpec("core"),) * (n_params + n_outs)
        out_specs = (PartitionSpec("core"),) * n_outs
        self.donate = tuple(range(n_params, n_params + n_outs))
        self.fn = jax.jit(
            shard_map(
                _body,
                mesh=self.mesh,
                in_specs=in_specs,
                out_specs=out_specs,
                check_rep=False,
            ),
            donate_argnums=self.donate,
            keep_unused=True,
        )

    def zeros(self):
        return [
            np.zeros((N_CORES * a.shape[0], *a.shape[1:]), a.dtype)
            for a in self.out_avals
        ]

    def prep(self, x, w, b):
        """Global (stacked-over-cores) input arrays, in in_names order.

        Core c = fc*ROW_SHARDS + rc gets rows rc*ROWS..(rc+1)*ROWS and
        feature columns fc*WIDTH..(fc+1)*WIDTH (blocks fc*NB..(fc+1)*NB).
        """
        x2 = np.asarray(x, dtype=np.float32).reshape(ROWS_TOTAL, WIDTH_TOTAL)
        w = np.asarray(w, dtype=np.float32)
        b = np.asarray(b, dtype=np.float32)
        xs, ws, bs = [], [], []
        for c in range(N_CORES):
            fc, rc = divmod(c, ROW_SHARDS)
            xs.append(x2[rc * ROWS:(rc + 1) * ROWS,
                         fc * WIDTH:(fc + 1) * WIDTH])
            ws.append(w[fc * NB:(fc + 1) * NB])
            bs.append(b[fc * NB:(fc + 1) * NB])
        per = {
            "x": np.ascontiguousarray(np.concatenate(xs, axis=0)),
            "w": np.ascontiguousarray(np.concatenate(ws, axis=0)),
            "b": np.ascontiguousarray(np.concatenate(bs, axis=0)),
        }
        return [per[n] for n in self.in_names]

    def unshard_y(self, y_global):
        """[N_CORES*ROWS, WIDTH] stacked shards -> [ROWS_TOTAL, WIDTH_TOTAL]."""
        out = np.empty((ROWS_TOTAL, WIDTH_TOTAL), np.float32)
        for c in range(N_CORES):
            fc, rc = divmod(c, ROW_SHARDS)
            out[rc * ROWS:(rc + 1) * ROWS, fc * WIDTH:(fc + 1) * WIDTH] = \
                y_global[c * ROWS:(c + 1) * ROWS]
        return out

    def __call__(self, ins, zeros):
        outs = self.fn(*ins, *zeros)
        return dict(zip(self.out_names, outs))


_RUNNER = None


def _get_runner():
    global _RUNNER
    if _RUNNER is None:
        _RUNNER = _Runner()
    return _RUNNER


def kernel(x, w, b):
    r = _get_runner()
    outs = r(r.prep(x, w, b), r.zeros())
    y = r.unshard_y(np.asarray(outs["y"]))
    return y.reshape(4, 4096, WIDTH_TOTAL)


# revision 14
# speedup vs baseline: 420.0566x; 1.0047x over previous
"""Block-diagonal linear for TRN2, 8 NeuronCores.

y = concat_h(x_h @ w_h + b_h) with x:[4,4096,4096] split into 16 blocks of
256 features; w:[16,256,256]; b:[16,256].

Sharding: data-parallel over rows. x is reshaped to [16384, 4096] and each
core takes 2048 contiguous rows; w and b are replicated. Zero communication.
(A hybrid rows-x-feature-blocks split was measured ~25us slower despite
halving replicated-w traffic: the narrower per-core tiles shrink DMA runs
and make the y drain lag the tile stream.)

Per-core kernel (Tile framework), v3:
  - Matmuls run as float32r (single-pass PE streaming, 1 cyc/row at N=256 —
    4x the fp32 LOW_HIGH dual-pass rate; operands rounded to fp32r costs
    ~1.5e-4 relative error, well inside the 2e-2 gate).
  - w is staged in SBUF as [128, 8, 2, 256] fp32r (contraction on partitions)
    via the gpsimd SWDGE queue: its gather pattern needs ~2048 descriptors,
    which would head-of-line block an HWDGE sequencer (ACT blocked ~25us
    measured); Q7 generates them off the critical path.
  - b is broadcast across partitions once via gpsimd partition_broadcast.
  - Per 128-row tile: x DMAs in as 2 half-tiles (8KB/partition descriptors)
    on the SP HWDGE ring. Per group g of 4 feature-chunks: PE-transposes
    (fp32, bit-exact) land in PSUM, ACT evicts+rounds them to an fp32r xT
    buffer, and the 4 fp32r matmuls for blocks 2g,2g+1 consume them. The
    emit order is software-pipelined (transposes of group g+1 issue before
    the matmuls of group g) so the PE covers the ACT eviction latency with
    transpose work instead of stalling.
  - DVE adds bias while evicting PSUM->SBUF; y streams out per half-tile on
    the gpsimd (SWDGE) queue so output drains never head-of-line block the
    SP x-loads or the ACT evictions.
"""

import numpy as np

import concourse.bacc as bacc
import concourse.mybir as mybir
from concourse import bass2jax, tile
from concourse.masks import make_identity

N_CORES = 8
ROW_SHARDS = 8
FEAT_SHARDS = 1
ROWS_TOTAL = 4 * 4096
WIDTH_TOTAL = 4096
NB_TOTAL = 16
ROWS = ROWS_TOTAL // ROW_SHARDS  # 2048 rows per core
WIDTH = WIDTH_TOTAL // FEAT_SHARDS  # 4096 features per core
NB = NB_TOTAL // FEAT_SHARDS  # 16 blocks per core
BW = 256  # block width
P = 128
M_TILES = ROWS // P  # 16
NCH = WIDTH // P  # 32 feature chunks of 128
GROUPS = NCH // 4  # 8 groups of 4 chunks per tile

FP32 = mybir.dt.float32
FP32R = mybir.dt.float32r


def _build():
    nc = bacc.Bacc(None, target_bir_lowering=False, debug=False)
    x = nc.dram_tensor("x", [ROWS, WIDTH], FP32, kind="ExternalInput")
    w = nc.dram_tensor("w", [NB, BW, BW], FP32, kind="ExternalInput")
    b = nc.dram_tensor("b", [NB, BW], FP32, kind="ExternalInput")
    y = nc.dram_tensor("y", [ROWS, WIDTH], FP32, kind="ExternalOutput")

    with tile.TileContext(nc) as tc:
        with (
            tc.tile_pool(name="const", bufs=1) as const_pool,
            tc.tile_pool(name="xpool", bufs=3) as x_pool,
            tc.tile_pool(name="xtpool", bufs=2) as xt_pool,
            tc.tile_pool(name="ypool", bufs=2) as y_pool,
            tc.tile_pool(name="pt", bufs=3, space="PSUM") as psum_t,
            tc.tile_pool(name="py", bufs=5, space="PSUM") as psum_y,
        ):
            ident = const_pool.tile([P, P], FP32)
            make_identity(nc, ident[:])

            # Bias broadcast across partitions: b_rep[p, h, j] = b[h, j].
            b_lin = const_pool.tile([1, NB, BW], FP32)
            nc.gpsimd.dma_start(
                b_lin[:], b.rearrange("(one h) j -> one h j", one=1)
            )

            # w_sb[p, h, ic, j] = w[h, ic*128+p, j] (fp32r view: PE rounds
            # internally, bits are plain fp32). SWDGE queue: the gather
            # pattern's ~2048 descriptors would head-block an HWDGE ring.
            w_sb = const_pool.tile([P, NB, 2, BW], FP32R)
            w_re = w.rearrange("h (ic p) j -> p h ic j", p=P)
            for q in range(4):
                nc.gpsimd.dma_start(
                    w_sb[:, 4 * q:4 * q + 4],
                    w_re[:, 4 * q:4 * q + 4].bitcast(FP32R),
                )
            b_rep = const_pool.tile([P, NB, BW], FP32)
            nc.gpsimd.partition_broadcast(
                b_rep[:].rearrange("p h j -> p (h j)"),
                b_lin[:].rearrange("o h j -> o (h j)"),
            )

            x_rows = x.rearrange("(t p) i -> t p i", p=P)
            y_rows = y.rearrange("(t p) i -> t p i", p=P)
            HW = WIDTH // 2  # half-tile width

            for mi in range(M_TILES):
                x_t = x_pool.tile([P, WIDTH], FP32)
                for hf in range(2):
                    nc.sync.dma_start(
                        x_t[:, hf * HW:(hf + 1) * HW],
                        x_rows[mi][:, hf * HW:(hf + 1) * HW],
                    )

                xT = xt_pool.tile([P, NCH, P], FP32R)
                y_t = y_pool.tile([P, NB, BW], FP32)
                y_flat = y_t[:].rearrange("p h j -> p (h j)")

                # Software pipeline over the groups: transposes of group g+1
                # are emitted before the matmuls of group g so the PE has
                # transpose work while ACT evicts group g's PSUM.
                pts = {}

                def do_transposes(g):
                    pt = psum_t.tile([P, 4, P], FP32, tag="pt")
                    for k in range(4):
                        c = 4 * g + k
                        nc.tensor.transpose(
                            pt[:, k, :], x_t[:, c * P:(c + 1) * P], ident[:]
                        )
                    pts[g] = pt

                do_transposes(0)
                for g in range(GROUPS):
                    # ACT: evict + round group g's transposed chunks.
                    nc.scalar.copy(xT[:, 4 * g:4 * g + 4, :], pts.pop(g)[:])
                    if g < GROUPS - 1:
                        do_transposes(g + 1)

                    py = psum_y.tile([P, 2, BW], FP32)
                    for u in range(2):
                        h = 2 * g + u
                        nc.tensor.matmul(
                            py[:, u, :], xT[:, 2 * h, :], w_sb[:, h, 0, :],
                            start=True, stop=False,
                        )
                        nc.tensor.matmul(
                            py[:, u, :], xT[:, 2 * h + 1, :], w_sb[:, h, 1, :],
                            start=False, stop=True,
                        )
                    nc.vector.tensor_add(
                        y_t[:, 2 * g:2 * g + 2, :],
                        py[:],
                        b_rep[:, 2 * g:2 * g + 2, :],
                    )
                    if g == GROUPS // 2 - 1 or g == GROUPS - 1:
                        hf = 0 if g == GROUPS // 2 - 1 else 1
                        nc.gpsimd.dma_start(
                            y_rows[mi][:, hf * HW:(hf + 1) * HW],
                            y_flat[:, hf * HW:(hf + 1) * HW],
                        )

    nc.compile()
    return nc


class _Runner:
    """Compile once, keep the jitted SPMD executable for reuse."""

    def __init__(self):
        import jax
        from jax.experimental.shard_map import shard_map
        from jax.sharding import Mesh, PartitionSpec

        self.jax = jax
        nc = _build()
        self.nc = nc
        bass2jax.install_neuronx_cc_hook()

        assert nc.dbg_addr is None
        part_name = (
            nc.partition_id_tensor.name if nc.partition_id_tensor else None
        )
        in_names, out_names, out_avals = [], [], []
        for alloc in nc.m.functions[0].allocations:
            if not isinstance(alloc, mybir.MemoryLocationSet):
                continue
            name = alloc.memorylocations[0].name
            if alloc.kind == "ExternalInput":
                if name != part_name:
                    in_names.append(name)
            elif alloc.kind == "ExternalOutput":
                out_names.append(name)
                out_avals.append(
                    jax.core.ShapedArray(
                        tuple(alloc.tensor_shape), mybir.dt.np(alloc.dtype)
                    )
                )
        self.in_names = list(in_names)
        self.out_names = out_names
        self.out_avals = out_avals
        n_params = len(in_names)
        n_outs = len(out_names)
        all_names = in_names + out_names
        if part_name is not None:
            all_names = all_names + [part_name]

        def _body(*args):
            operands = list(args)
            if part_name is not None:
                operands.append(bass2jax.partition_id_tensor())
            outs = bass2jax._bass_exec_p.bind(
                *operands,
                out_avals=tuple(out_avals),
                in_names=tuple(all_names),
                out_names=tuple(out_names),
                lowering_input_output_aliases=(),
                sim_require_finite=True,
                sim_require_nnan=True,
                nc=nc,
            )
            return tuple(outs)

        devices = jax.devices()[:N_CORES]
        assert len(devices) == N_CORES
        self.mesh = Mesh(np.asarray(devices), ("core",))
        in_specs = (PartitionSpec("core"),) * (n_params + n_outs)
        out_specs = (PartitionSpec("core"),) * n_outs
        self.donate = tuple(range(n_params, n_params + n_outs))
        self.fn = jax.jit(
            shard_map(
                _body,
                mesh=self.mesh,
                in_specs=in_specs,
                out_specs=out_specs,
                check_rep=False,
            ),
            donate_argnums=self.donate,
            keep_unused=True,
        )

    def zeros(self):
        return [
            np.zeros((N_CORES * a.shape[0], *a.shape[1:]), a.dtype)
            for a in self.out_avals
        ]

    def prep(self, x, w, b):
        """Global (stacked-over-cores) input arrays, in in_names order.

        Core c = fc*ROW_SHARDS + rc gets rows rc*ROWS..(rc+1)*ROWS and
        feature columns fc*WIDTH..(fc+1)*WIDTH (blocks fc*NB..(fc+1)*NB).
        """
        x2 = np.asarray(x, dtype=np.float32).reshape(ROWS_TOTAL, WIDTH_TOTAL)
        w = np.asarray(w, dtype=np.float32)
        b = np.asarray(b, dtype=np.float32)
        xs, ws, bs = [], [], []
        for c in range(N_CORES):
            fc, rc = divmod(c, ROW_SHARDS)
            xs.append(x2[rc * ROWS:(rc + 1) * ROWS,
                         fc * WIDTH:(fc + 1) * WIDTH])
            ws.append(w[fc * NB:(fc + 1) * NB])
            bs.append(b[fc * NB:(fc + 1) * NB])
        per = {
            "x": np.ascontiguousarray(np.concatenate(xs, axis=0)),
            "w": np.ascontiguousarray(np.concatenate(ws, axis=0)),
            "b": np.ascontiguousarray(np.concatenate(bs, axis=0)),
        }
        return [per[n] for n in self.in_names]

    def unshard_y(self, y_global):
        """[N_CORES*ROWS, WIDTH] stacked shards -> [ROWS_TOTAL, WIDTH_TOTAL]."""
        out = np.empty((ROWS_TOTAL, WIDTH_TOTAL), np.float32)
        for c in range(N_CORES):
            fc, rc = divmod(c, ROW_SHARDS)
            out[rc * ROWS:(rc + 1) * ROWS, fc * WIDTH:(fc + 1) * WIDTH] = \
                y_global[c * ROWS:(c + 1) * ROWS]
        return out

    def __call__(self, ins, zeros):
        outs = self.fn(*ins, *zeros)
        return dict(zip(self.out_names, outs))


_RUNNER = None


def _get_runner():
    global _RUNNER
    if _RUNNER is None:
        _RUNNER = _Runner()
    return _RUNNER


def kernel(x, w, b):
    r = _get_runner()
    outs = r(r.prep(x, w, b), r.zeros())
    y = r.unshard_y(np.asarray(outs["y"]))
    return y.reshape(4, 4096, WIDTH_TOTAL)
